# revision 14
# baseline (speedup 1.0000x reference)
"""Trainium2 Bass kernel for nn_Attention_33054068310137 (8-core SPMD).

Math: per (head h, batch b):
    blend = (1-w_h) * cosine + w_h * cov        # [N, N]
    out_h = blend @ fv                          # [N, DH]
Both score matrices are low-rank outer products of per-row-rescaled
projections, so with
    A_q[n] = [fq[n]/|fq[n]| ; fq[n]-qm[n]]      # [N, 2*DH]
    S      = [Acos_k ; fk]^T @ fv  with a rank-1 (row-mean) correction
             on the bottom half:  S_bot = fk^T fv - 1 ox (km^T fv)
    S'     = diag([(1-w)*1 ; (w/DH)*1]) @ S
we get out_h = A_q @ S' without materializing the N x N matrices.

Sharding: rows (B*N = 4096) split across 8 cores (cores 0-3 batch 0,
cores 4-7 batch 1). Two ncfw collectives, both triggered as early as
possible:
  AR1: all-8 AllReduce (2KB bf16) of the global row-sum partials of the
       *LN-normalized* q,k. Computed straight from the raw input tiles
       plus per-row LN stats via tiny PE matmuls
       (sum_n rstd_n*x[n,:] + sum_n(-m_n*rstd_n)), i.e. without waiting
       for the normalize/transpose pipeline -- trigger at ~t=18us.
       The head-mean projection (@W_eff + bias) runs after the reduce.
  AR2: per-batch-group ([[0-3],[4-7]]) AllReduce of the S partials
       (bf16, 128KB). The normalize/transpose/projection pipeline and
       the weight-predictor MLP overlap the collective flights.
Weights are replicated; W_in/W_out are pre-cast to bf16 on the host
(they were cast on-device before, so numerics are unchanged).
"""

import os
import numpy as np

H, DH, B, N, D = 8, 64, 2, 2048, 512
EPS = 1e-5
N_CORES = 8
R = (B * N) // N_CORES  # rows per core = 512
P = 128                 # SBUF partitions

WARM_A = int(os.environ.get("BASSK_WARM_A", "60"))  # upfront PE clock ramp
WARM_B = int(os.environ.get("BASSK_WARM_B", "16"))  # pre-S bridge
WARM_C = int(os.environ.get("BASSK_WARM_C", "48"))  # pre-tail bridge
WARM_LN = int(os.environ.get("BASSK_WARM_LN", "0"))  # per-row-tile drumbeat

_CACHE = {}


def _build_program():
    import concourse.bacc as bacc
    import concourse.bass as bass
    import concourse.mybir as mybir
    import concourse.tile as tile
    from concourse.masks import make_identity
    from contextlib import ExitStack

    f32 = mybir.dt.float32
    bf16 = mybir.dt.bfloat16
    CDT = bf16
    AX = mybir.AxisListType
    OP = mybir.AluOpType
    AF = mybir.ActivationFunctionType

    nc = bacc.Bacc("TRN2", target_bir_lowering=False, debug=False,
                   enable_asserts=True, num_devices=N_CORES)

    # ---- external I/O (per-core shapes) ----
    q_ext = nc.dram_tensor("q", [R, D], f32, kind="ExternalInput")
    k_ext = nc.dram_tensor("k", [R, D], f32, kind="ExternalInput")
    v_ext = nc.dram_tensor("v", [R, D], f32, kind="ExternalInput")
    winT_ext = nc.dram_tensor("W_inT", [D, D], bf16, kind="ExternalInput")    # [d, j]
    woutT_ext = nc.dram_tensor("W_outT", [D, D], bf16, kind="ExternalInput")  # [j, dcol]
    lng_ext = nc.dram_tensor("ln_g", [D], f32, kind="ExternalInput")
    lnb_ext = nc.dram_tensor("ln_b", [D], f32, kind="ExternalInput")
    bout_ext = nc.dram_tensor("b_out", [D], f32, kind="ExternalInput")
    w1T_ext = nc.dram_tensor("wp_w1T", [2 * DH, DH], f32, kind="ExternalInput")
    b1_ext = nc.dram_tensor("wp_b1", [DH], f32, kind="ExternalInput")
    wlg_ext = nc.dram_tensor("wp_ln_g", [DH], f32, kind="ExternalInput")
    wlb_ext = nc.dram_tensor("wp_ln_b", [DH], f32, kind="ExternalInput")
    w2_ext = nc.dram_tensor("wp_w2", [DH], f32, kind="ExternalInput")
    b2_ext = nc.dram_tensor("wp_b2", [1], f32, kind="ExternalInput")
    out_ext = nc.dram_tensor("out", [D, R], f32, kind="ExternalOutput")  # transposed
    DEBUG = os.environ.get("BASSK_DEBUG", "0") == "1"
    if DEBUG:
        dbg_ar1 = nc.dram_tensor("dbg_ar1", [P, 8], mybir.dt.bfloat16, kind="ExternalOutput")
        dbg_feat = nc.dram_tensor("dbg_feat", [2 * DH, H], mybir.dt.bfloat16, kind="ExternalOutput")
        dbg_w = nc.dram_tensor("dbg_w", [H, 1], f32, kind="ExternalOutput")
        dbg_ssum = nc.dram_tensor("dbg_ssum", [P, H * DH], mybir.dt.bfloat16, kind="ExternalOutput")

    NT = R // P  # row tiles per core = 4
    ND = D // P  # feature tiles = 4

    def _patch_pool(pool):
        orig = pool.tile
        def tile_(shape, dtype, tag, **kw):
            return orig(shape, dtype, name=tag, tag=tag, **kw)
        pool.tile = tile_
        return pool

    with tile.TileContext(nc) as tc, ExitStack() as ctx:
        consts = _patch_pool(ctx.enter_context(tc.tile_pool(name="consts", bufs=1)))
        wp = _patch_pool(ctx.enter_context(tc.tile_pool(name="wp", bufs=1)))
        work = _patch_pool(ctx.enter_context(tc.tile_pool(name="work", bufs=4)))
        keep = _patch_pool(ctx.enter_context(tc.tile_pool(name="keep", bufs=1)))
        psum = _patch_pool(ctx.enter_context(tc.tile_pool(name="psum", bufs=1, space="PSUM")))
        pss = _patch_pool(ctx.enter_context(tc.tile_pool(name="pss", bufs=1, space="PSUM")))
        dram = _patch_pool(ctx.enter_context(tc.tile_pool(name="dram", bufs=1, space="DRAM")))

        # ---------------- constants ----------------
        ident = consts.tile([P, P], CDT, tag="ident")
        make_identity(nc, ident[:])
        ones_row = consts.tile([1, P], CDT, tag="ones_row")      # K=1 bias matmul lhsT
        nc.vector.memset(ones_row[:], 1.0)
        ones_row_f = consts.tile([1, P], f32, tag="ones_row_f")
        nc.vector.memset(ones_row_f[:], 1.0)
        ones_col_bf = consts.tile([P, 1], CDT, tag="ones_col_bf")
        nc.vector.memset(ones_col_bf[:], 1.0)
        neg1_row64 = consts.tile([1, DH], CDT, tag="neg1_row64")
        nc.vector.memset(neg1_row64[:], -1.0)
        eps_t = consts.tile([P, 1], f32, tag="eps_t")
        nc.vector.memset(eps_t[:], EPS)
        ind_top = consts.tile([1, P], bf16, tag="ind_top")
        nc.vector.memset(ind_top[:], 0.0)
        nc.vector.memset(ind_top[:, 0:DH], 1.0)
        ind_bot = consts.tile([1, P], bf16, tag="ind_bot")
        nc.vector.memset(ind_bot[:], 0.0)
        nc.vector.memset(ind_bot[:, DH:P], 1.0)
        one1 = consts.tile([1, 1], CDT, tag="one1")
        nc.vector.memset(one1[:], 1.0)
        ident8 = consts.tile([H, H], f32, tag="ident8")
        make_identity(nc, ident8[:])

        # PE clock warm-up: dense matmuls during the initial DMA window
        warm_sink = consts.tile([1, 1], f32, tag="warm_sink")
        if WARM_A:
            warm_ps = psum.tile([P, P], f32, tag="trps", bufs=2)
            for wi in range(WARM_A):
                nc.tensor.matmul(warm_ps[:], ident[:], ident[:],
                                 start=True, stop=True)
            nc.vector.tensor_copy(warm_sink[:], warm_ps[0:1, 0:1])

        # ---------------- phase 1: LN + normalize + AR1 row-sums ----------
        def ln_stats(x_sb):
            """bn stats -> r2 [P, 2] = [rstd | -m*rstd]."""
            st6 = work.tile([P, 6], f32, tag="st6")
            nc.vector.bn_stats(st6[:], x_sb[:])
            mv = work.tile([P, 2], f32, tag="mv")
            nc.vector.bn_aggr(mv[:], st6[:])
            r2 = work.tile([P, 2], f32, tag="r2", bufs=6)
            nc.scalar.activation(r2[:, 0:1], mv[:, 1:2], AF.Sqrt, bias=eps_t[:])
            nc.vector.reciprocal(r2[:, 0:1], r2[:, 0:1])
            nc.vector.tensor_scalar(r2[:, 1:2], mv[:, 0:1], scalar1=-1.0,
                                    scalar2=r2[:, 0:1], op0=OP.mult, op1=OP.mult)
            return r2

        def normalize(x_sb, r2, rt, name):
            xn = keep.tile([P, D], CDT, tag=f"xn_{name}")
            if rt % 2 == 0:
                nc.scalar.activation(xn[:], x_sb[:], AF.Identity,
                                     bias=r2[:, 1:2], scale=r2[:, 0:1])
            else:
                nc.vector.tensor_scalar(xn[:], x_sb[:], scalar1=r2[:, 0:1],
                                        scalar2=r2[:, 1:2],
                                        op0=OP.mult, op1=OP.add)
            return xn

        ps_xbq = pss.tile([P, ND], f32, tag="pssA")
        ps_xbk = pss.tile([P, ND], f32, tag="pssB")
        xn_q, xn_k = [], []

        def phase1(x_ext, name, dmae, xn_list, ps_xb):
            for rt in range(NT):
                x = work.tile([P, D], f32, tag=f"x_{name}", bufs=3)
                dmae.dma_start(x[:], x_ext[rt * P:(rt + 1) * P, :])
                r2 = ln_stats(x)
                xn = normalize(x, r2, rt, f"{name}{rt}")
                xn_list.append(xn)
                for di in range(ND):
                    nc.tensor.matmul(
                        ps_xb[:, di:di + 1],
                        xn[:, di * P:(di + 1) * P],
                        ones_col_bf[:],
                        start=(rt == 0), stop=(rt == NT - 1))

        phase1(q_ext, "q", nc.sync, xn_q, ps_xbq)
        phase1(k_ext, "k", nc.scalar, xn_k, ps_xbk)

        ar1_sb = keep.tile([P, 2 * ND], bf16, tag="ar1_sb")
        nc.vector.tensor_scalar_mul(ar1_sb[:, 0:ND], ps_xbq[:], 1.0 / (B * N))
        nc.scalar.activation(ar1_sb[:, ND:2 * ND], ps_xbk[:], AF.Identity,
                             scale=1.0 / (B * N))
        AR1EL = P * 2 * ND
        ar1_in = dram.tile([AR1EL], bf16, tag="ar1_in")
        ar1_out = dram.tile([AR1EL], bf16, tag="ar1_out")
        nc.sync.dma_start(ar1_in[:].rearrange("(p f) -> p f", p=P), ar1_sb[:])
        nc.gpsimd.collective_compute(
            "AllReduce", OP.add,
            replica_groups=[list(range(N_CORES))],
            ins=[ar1_in.opt()], outs=[ar1_out.opt()])

        # ---------------- weights (overlap AR1 flight) ----------------
        winT = []
        weff = []
        bsl_bf = []
        for di in range(ND):
            wt = wp.tile([P, D], bf16, tag=f"winT{di}")
            nc.scalar.dma_start(wt[:], winT_ext[di * P:(di + 1) * P, :])
            winT.append(wt)
            g = wp.tile([P, 1], f32, tag=f"gsl{di}")
            nc.scalar.dma_start(g[:], lng_ext[di * P:(di + 1) * P].unsqueeze(1))
            b = wp.tile([P, 1], f32, tag=f"bsl{di}")
            nc.scalar.dma_start(b[:], lnb_ext[di * P:(di + 1) * P].unsqueeze(1))
            bb = wp.tile([P, 1], bf16, tag=f"bslb{di}")
            nc.vector.tensor_copy(bb[:], b[:])
            bsl_bf.append(bb)
            we = wp.tile([P, D], CDT, tag=f"weff{di}")
            nc.vector.tensor_scalar_mul(we[:], wt[:], g[:])  # W_in^T * g
            weff.append(we)

        # bias_row[1, j] = ln_b @ W_in^T  (rank-1 LN-bias term)
        bias_ps = pss.tile([1, D], f32, tag="pssD")
        for di in range(ND):
            nc.tensor.matmul(bias_ps[:], bsl_bf[di][:], winT[di][:],
                             start=(di == 0), stop=(di == ND - 1))
        bias_row = wp.tile([1, D], CDT, tag="bias_row")
        nc.scalar.copy(bias_row[:], bias_ps[:])
        # bias_rowT [DH, H]: bias_rowT[c, h] = bias_row[h*DH+c]
        brT_ps = pss.tile([DH, H], f32, tag="pssE")
        for di in range(ND):
            for h in range(H):
                nc.tensor.matmul(
                    brT_ps[:, h:h + 1],
                    winT[di][:, h * DH:(h + 1) * DH],
                    bsl_bf[di][:],
                    start=(di == 0), stop=(di == ND - 1))
        bias_rT = wp.tile([DH, H], f32, tag="bias_rT")
        nc.vector.tensor_copy(bias_rT[:], brT_ps[:])

        # weight-predictor weights
        w1T = wp.tile([2 * DH, DH], f32, tag="w1T")
        nc.gpsimd.dma_start(w1T[:], w1T_ext[:])
        w1T_bf = wp.tile([2 * DH, DH], bf16, tag="w1T_bf")
        nc.vector.tensor_copy(w1T_bf[:], w1T[:])
        b1_rep = wp.tile([H, DH], f32, tag="b1_rep")
        nc.gpsimd.dma_start(b1_rep[:], b1_ext[None, :].to_broadcast((H, DH)))
        wlg_rep = wp.tile([H, DH], f32, tag="wlg_rep")
        nc.gpsimd.dma_start(wlg_rep[:], wlg_ext[None, :].to_broadcast((H, DH)))
        wlb_rep = wp.tile([H, DH], f32, tag="wlb_rep")
        nc.gpsimd.dma_start(wlb_rep[:], wlb_ext[None, :].to_broadcast((H, DH)))
        w2_rep = wp.tile([H, DH], f32, tag="w2_rep")
        nc.gpsimd.dma_start(w2_rep[:], w2_ext[None, :].to_broadcast((H, DH)))
        b2_col = wp.tile([H, 1], f32, tag="b2_col")
        nc.gpsimd.dma_start(b2_col[:], b2_ext[None, :].to_broadcast((H, 1)))

        # W_out (tail-only weights)
        woutT = []
        bout = []
        for jt in range(ND):
            wo = wp.tile([P, D], CDT, tag=f"woutT{jt}")
            nc.gpsimd.dma_start(wo[:], woutT_ext[jt * P:(jt + 1) * P, :])
            woutT.append(wo)
            bo = wp.tile([P, 1], f32, tag=f"bout{jt}")
            nc.gpsimd.dma_start(bo[:], bout_ext[jt * P:(jt + 1) * P].unsqueeze(1))
            bout.append(bo)

        # ---------------- per-tensor pipeline helpers ----------------
        def transpose_xn(xn):
            if WARM_LN:
                wps = psum.tile([P, P], f32, tag="trps", bufs=2)
                for wi in range(WARM_LN):
                    nc.tensor.matmul(wps[:], ident[:], ident[:],
                                     start=True, stop=True)
            tr_ps = psum.tile([P, D], CDT, tag="trps", bufs=2)
            for di in range(ND):
                nc.tensor.transpose(
                    tr_ps[:, di * P:(di + 1) * P],
                    xn[:, di * P:(di + 1) * P], ident[:])
            return tr_ps

        def project(xnT_ps_view, name, rt, act_copy):
            """fx[rt] = xn @ (W_in*g)^T + ln_b @ W_in^T; returns psum tile."""
            # xnT_ps_view: [P(d), (di, P rows)] psum from transpose; must be
            # copied to SBUF first (matmul lhsT reads SBUF).
            xnT = work.tile([P, D], CDT, tag="xnT_sb", bufs=6)
            if rt % 2 == 0:
                nc.vector.tensor_copy(xnT[:], xnT_ps_view)
            else:
                nc.scalar.copy(xnT[:], xnT_ps_view)
            pj = psum.tile([P, D], f32, tag="projps", bufs=2)
            for di in range(ND):
                nc.tensor.matmul(
                    pj[:], xnT[:, di * P:(di + 1) * P], weff[di][:],
                    start=(di == 0), stop=False)
            nc.tensor.matmul(pj[:], ones_row[:], bias_row[:],
                             start=False, stop=True)
            return pj

        # ---- k path: transpose, project, A_k = [cos | centered] ----
        def rowstats_A(pj, rt, keep_A_tag=None):
            """fx copy + per-head inv-norm / mean; A = [cos | centered]."""
            fx = work.tile([P, D], CDT, tag="fx", bufs=4)
            if rt % 2 == 0:
                nc.scalar.copy(fx[:], pj[:])
            else:
                nc.vector.tensor_copy(fx[:], pj[:])
            fx3 = fx[:].rearrange("p (h c) -> p h c", h=H)
            sqh = work.tile([P, D], CDT, tag="sqh")
            nc.scalar.activation(sqh[:], pj[:], AF.Square)
            qn2 = work.tile([P, H], f32, tag="qn2")
            nc.vector.reduce_sum(
                qn2[:], sqh[:].rearrange("p (h c) -> p h c", h=H), axis=AX.X)
            qsum = work.tile([P, H], f32, tag="qsum")
            nc.vector.reduce_sum(qsum[:], fx3, axis=AX.X)
            invn = work.tile([P, H], f32, tag="invn")
            nc.scalar.activation(invn[:], qn2[:], AF.Sqrt)
            nc.vector.reciprocal(invn[:], invn[:])
            hmean = work.tile([P, H], f32, tag="hmean")
            nc.vector.tensor_scalar_mul(hmean[:], qsum[:], 1.0 / DH)
            if keep_A_tag is not None:
                A = keep.tile([P, 2 * D], CDT, tag=keep_A_tag)
            else:
                A = work.tile([P, 2 * D], CDT, tag="A_q", bufs=2)
            A4 = A[:].rearrange("p (h c) -> p h c", h=H)
            nc.vector.tensor_tensor(
                A4[:, :, 0:DH], fx3,
                invn[:, :, None].broadcast_to((P, H, DH)), op=OP.mult)
            nc.vector.tensor_tensor(
                A4[:, :, DH:2 * DH], fx3,
                hmean[:, :, None].broadcast_to((P, H, DH)), op=OP.subtract)
            return A

        Ak = []
        for rt in range(NT):
            tr_ps = transpose_xn(xn_k[rt])
            pj = project(tr_ps[:], "k", rt, act_copy=True)
            Ak.append(rowstats_A(pj, rt, keep_A_tag=f"A_k{rt}"))

        # ---- v path: full LN + project -> fv tiles ----
        fv_tiles = []
        for rt in range(NT):
            xv = work.tile([P, D], f32, tag="x_v", bufs=3)
            nc.sync.dma_start(xv[:], v_ext[rt * P:(rt + 1) * P, :])
            r2 = ln_stats(xv)
            xnv = normalize(xv, r2, rt, f"v{rt}")
            tr_ps = transpose_xn(xnv)
            pj = project(tr_ps[:], "v", rt, act_copy=True)
            fv = keep.tile([P, D], CDT, tag=f"fv{rt}")
            if rt % 2 == 0:
                nc.scalar.copy(fv[:], pj[:])
            else:
                nc.vector.tensor_copy(fv[:], pj[:])
            fv_tiles.append(fv)

        if WARM_B:
            warm3_ps = psum.tile([P, P], f32, tag="trps", bufs=2)
            for wi in range(WARM_B):
                nc.tensor.matmul(warm3_ps[:], ident[:], fv_tiles[3][:, 0:P],
                                 start=True, stop=True)
            nc.vector.tensor_copy(warm_sink[:], warm3_ps[0:1, 0:1])

        # ---- S partials ----
        s_ps = pss.tile([P, H * DH], f32, tag="pssA")
        for h in range(H):
            for rt in range(NT):
                nc.tensor.matmul(
                    s_ps[:, h * DH:(h + 1) * DH],
                    Ak[rt][:, h * 2 * DH:(h + 1) * 2 * DH],
                    fv_tiles[rt][:, h * DH:(h + 1) * DH],
                    start=(rt == 0), stop=(rt == NT - 1))
        s_bf = keep.tile([P, H * DH], bf16, tag="s_bf")
        nc.vector.tensor_copy(s_bf[:], s_ps[:])
        SEL = P * H * DH
        ar2_in = dram.tile([SEL], bf16, tag="ar2_in")
        ar2_out = dram.tile([SEL], bf16, tag="ar2_out")
        nc.sync.dma_start(ar2_in[:].rearrange("(p f) -> p f", p=P), s_bf[:])
        nc.gpsimd.collective_compute(
            "AllReduce", OP.add,
            replica_groups=[[0, 1, 2, 3], [4, 5, 6, 7]],
            ins=[ar2_in.opt()], outs=[ar2_out.opt()])

        # ---- q path (overlaps AR2) ----
        AqT = []
        for rt in range(NT):
            tr_ps = transpose_xn(xn_q[rt])
            pj = project(tr_ps[:], "q", rt, act_copy=True)
            A = rowstats_A(pj, rt)
            for h in range(H):
                if rt == 0:
                    at = keep.tile([P, R], CDT, tag=f"AqT{h}")
                    AqT.append(at)
                aq_ps = psum.tile([P, P], CDT, tag="trps", bufs=2)
                nc.tensor.transpose(
                    aq_ps[:], A[:, h * 2 * DH:(h + 1) * 2 * DH], ident[:])
                if h % 2 == 0:
                    nc.vector.tensor_copy(AqT[h][:, rt * P:(rt + 1) * P], aq_ps[:])
                else:
                    nc.scalar.copy(AqT[h][:, rt * P:(rt + 1) * P], aq_ps[:])

        # ---- weight-predictor MLP (needs AR1; overlaps AR2 flight) ----
        xg_sb = keep.tile([P, 2 * ND], bf16, tag="xg_sb")
        nc.scalar.dma_start(xg_sb[:], ar1_out[:].rearrange("(p f) -> p f", p=P))
        featq_ps = pss.tile([DH, H], f32, tag="pssD")
        featk_ps = pss.tile([DH, H], f32, tag="pssE")
        for di in range(ND):
            for h in range(H):
                nc.tensor.matmul(
                    featq_ps[:, h:h + 1],
                    weff[di][:, h * DH:(h + 1) * DH],
                    xg_sb[:, di:di + 1],
                    start=(di == 0), stop=(di == ND - 1))
                nc.tensor.matmul(
                    featk_ps[:, h:h + 1],
                    weff[di][:, h * DH:(h + 1) * DH],
                    xg_sb[:, ND + di:ND + di + 1],
                    start=(di == 0), stop=(di == ND - 1))
        featT = keep.tile([2 * DH, H], bf16, tag="featT")
        nc.vector.tensor_tensor(featT[0:DH, :], featq_ps[:], bias_rT[:], op=OP.add)
        nc.vector.tensor_tensor(featT[DH:2 * DH, :], featk_ps[:], bias_rT[:], op=OP.add)

        hid_ps = pss.tile([H, DH], f32, tag="pssD")
        nc.tensor.matmul(hid_ps[:], featT[:], w1T_bf[:], start=True, stop=True)
        hid = keep.tile([H, DH], f32, tag="hid")
        nc.vector.tensor_tensor(hid[:], hid_ps[:], b1_rep[:], op=OP.add)
        hst6 = keep.tile([H, 6], f32, tag="hst6")
        nc.vector.bn_stats(hst6[:], hid[:])
        hmv = keep.tile([H, 2], f32, tag="hmv")
        nc.vector.bn_aggr(hmv[:], hst6[:])
        hrstd = keep.tile([H, 1], f32, tag="hrstd")
        nc.scalar.activation(hrstd[:], hmv[:, 1:2], AF.Sqrt, bias=eps_t[0:H, :])
        nc.vector.reciprocal(hrstd[:], hrstd[:])
        hln = keep.tile([H, DH], f32, tag="hln")
        nc.vector.tensor_scalar(hln[:], hid[:], scalar1=hmv[:, 0:1],
                                scalar2=hrstd[:], op0=OP.subtract, op1=OP.mult)
        nc.vector.tensor_tensor(hln[:], hln[:], wlg_rep[:], op=OP.mult)
        nc.vector.tensor_tensor(hln[:], hln[:], wlb_rep[:], op=OP.add)
        nc.scalar.activation(hln[:], hln[:], AF.Relu)
        lscr = keep.tile([H, DH], f32, tag="lscr")
        nc.vector.tensor_tensor(lscr[:], hln[:], w2_rep[:], op=OP.mult)
        logit = keep.tile([H, 1], f32, tag="logit")
        nc.vector.reduce_sum(logit[:], lscr[:], axis=AX.X)
        wcol = keep.tile([H, 1], f32, tag="wcol")
        nc.scalar.activation(wcol[:], logit[:], AF.Sigmoid, bias=b2_col[:])
        wr_ps = pss.tile([1, H], f32, tag="pssE")
        nc.tensor.transpose(wr_ps[:], wcol[:], ident8[:])
        wrow = keep.tile([1, H], f32, tag="wrow")
        nc.vector.tensor_copy(wrow[:], wr_ps[:])
        omw = keep.tile([1, H], bf16, tag="omw")
        nc.vector.tensor_scalar(omw[:], wrow[:], scalar1=-1.0, scalar2=1.0,
                                op0=OP.mult, op1=OP.add)
        wdh = keep.tile([1, H], bf16, tag="wdh")
        nc.vector.tensor_scalar_mul(wdh[:], wrow[:], 1.0 / DH)
        wsc_ps = pss.tile([P, H], f32, tag="pssB")
        nc.tensor.matmul(wsc_ps[:], ind_top[:], omw[:], start=True, stop=False)
        nc.tensor.matmul(wsc_ps[:], ind_bot[:], wdh[:], start=False, stop=True)
        wsc = keep.tile([P, H], bf16, tag="wsc")
        nc.vector.tensor_copy(wsc[:], wsc_ps[:])

        # ---- S readback, blend-scale, final projection ----
        s_sum = keep.tile([P, H * DH], bf16, tag="s_sum")
        HSEL = SEL // 2
        nc.sync.dma_start(
            s_sum[0:P // 2, :], ar2_out[0:HSEL].rearrange("(p f) -> p f", p=P // 2))
        nc.scalar.dma_start(
            s_sum[P // 2:P, :],
            ar2_out[HSEL:SEL].rearrange("(p f) -> p f", p=P // 2))
        if WARM_C:
            warm2_ps = pss.tile([P, P], f32, tag="pssA")
            for wi in range(WARM_C):
                nc.tensor.matmul(warm2_ps[:], ident[:], ident[:],
                                 start=True, stop=True)
            nc.vector.tensor_copy(warm_sink[:], warm2_ps[0:1, 0:1])
        if DEBUG:
            nc.gpsimd.dma_start(dbg_ar1[:], xg_sb[:])
            nc.gpsimd.dma_start(dbg_feat[:], featT[:])
            nc.gpsimd.dma_start(dbg_w[:], wcol[:])
            nc.gpsimd.dma_start(dbg_ssum[:], s_sum[:])
        s_sc = keep.tile([P, H * DH], CDT, tag="s_sc")
        nc.vector.tensor_tensor(
            s_sc[:].rearrange("p (h c) -> p h c", h=H),
            s_sum[:].rearrange("p (h c) -> p h c", h=H),
            wsc[:, :, None].broadcast_to((P, H, DH)), op=OP.mult)

        foutT = []
        for jt in range(ND):
            ft = keep.tile([P, R], CDT, tag=f"foutT{jt}")
            foutT.append(ft)
        for h in range(H):
            m_ps = psum.tile([DH, R], f32, tag="projps", bufs=2)
            nc.tensor.matmul(m_ps[:], s_sc[:, h * DH:(h + 1) * DH], AqT[h][:],
                             start=True, stop=True)
            dst = foutT[h // 2][(h % 2) * DH:(h % 2) * DH + DH, :]
            if h % 2 == 0:
                nc.scalar.copy(dst, m_ps[:])
            else:
                nc.vector.tensor_copy(dst, m_ps[:])

        _ldq = [nc.sync, nc.scalar]
        for dt_ in range(ND):
            o_ps = psum.tile([P, R], f32, tag="projps", bufs=2)
            for jt in range(ND):
                nc.tensor.matmul(
                    o_ps[:], woutT[jt][:, dt_ * P:(dt_ + 1) * P], foutT[jt][:],
                    start=(jt == 0), stop=(jt == ND - 1))
            o_sb = work.tile([P, R], f32, tag="o_sb")
            if dt_ % 2 == 0:
                nc.scalar.activation(o_sb[:], o_ps[:], AF.Identity,
                                     bias=bout[dt_][:], scale=1.0)
            else:
                nc.vector.tensor_scalar_add(o_sb[:], o_ps[:], bout[dt_][:])
            _ldq[dt_ % 2].dma_start(out_ext[dt_ * P:(dt_ + 1) * P, :], o_sb[:])

    nc.finalize()
    return nc


def _get_program():
    if "nc" not in _CACHE:
        _CACHE["nc"] = _build_program()
    return _CACHE["nc"]


def _make_in_maps(inputs):
    import ml_dtypes
    bf = ml_dtypes.bfloat16
    q = np.ascontiguousarray(np.asarray(inputs["q"], np.float32).reshape(B * N, D))
    k = np.ascontiguousarray(np.asarray(inputs["k"], np.float32).reshape(B * N, D))
    v = np.ascontiguousarray(np.asarray(inputs["v"], np.float32).reshape(B * N, D))
    shared = {
        "W_inT": np.ascontiguousarray(np.asarray(inputs["W_in"], np.float32).T.astype(bf)),
        "W_outT": np.ascontiguousarray(np.asarray(inputs["W_out"], np.float32).T.astype(bf)),
        "ln_g": np.asarray(inputs["ln_g"], np.float32),
        "ln_b": np.asarray(inputs["ln_b"], np.float32),
        "b_out": np.asarray(inputs["b_out"], np.float32),
        "wp_w1T": np.ascontiguousarray(np.asarray(inputs["wp_w1"], np.float32).T),
        "wp_b1": np.asarray(inputs["wp_b1"], np.float32),
        "wp_ln_g": np.asarray(inputs["wp_ln_g"], np.float32),
        "wp_ln_b": np.asarray(inputs["wp_ln_b"], np.float32),
        "wp_w2": np.ascontiguousarray(np.asarray(inputs["wp_w2"], np.float32).reshape(DH)),
        "wp_b2": np.asarray(inputs["wp_b2"], np.float32).reshape(1),
    }
    in_maps = []
    for c in range(N_CORES):
        m = dict(shared)
        sl = slice(c * R, (c + 1) * R)
        m["q"] = np.ascontiguousarray(q[sl])
        m["k"] = np.ascontiguousarray(k[sl])
        m["v"] = np.ascontiguousarray(v[sl])
        in_maps.append(m)
    return in_maps


def _gather(results):
    out = np.empty((B * N, D), np.float32)
    for c in range(N_CORES):
        out[c * R:(c + 1) * R, :] = results[c]["out"].T
    return out.reshape(B, N, D)


def _run(inputs, trace=False, trace_cores=None):
    from concourse.bass_utils import run_bass_kernel_spmd
    nc = _get_program()
    in_maps = _make_in_maps(inputs)
    res = run_bass_kernel_spmd(
        nc, in_maps, core_ids=list(range(N_CORES)),
        trace=trace, trace_cores=trace_cores)
    return _gather(res.results), res


def kernel(**inputs) -> np.ndarray:
    out, _ = _run(inputs, trace=False)
    return out


def run_traced(inputs, trace_cores=None):
    return _run(inputs, trace=True, trace_cores=trace_cores)


# revision 17
# speedup vs baseline: 1.0547x; 1.0547x over previous
"""Trainium2 Bass kernel for nn_Attention_33054068310137 (8-core SPMD).

Math: per (head h, batch b):
    blend = (1-w_h) * cosine + w_h * cov        # [N, N]
    out_h = blend @ fv                          # [N, DH]
Both score matrices are low-rank outer products of per-row-rescaled
projections, so with
    A_q[n] = [fq[n]/|fq[n]| ; fq[n]-qm[n]]      # [N, 2*DH]
    S      = [Acos_k ; fk]^T @ fv  with a rank-1 (row-mean) correction
             on the bottom half:  S_bot = fk^T fv - 1 ox (km^T fv)
    S'     = diag([(1-w)*1 ; (w/DH)*1]) @ S
we get out_h = A_q @ S' without materializing the N x N matrices.

Sharding: rows (B*N = 4096) split across 8 cores (cores 0-3 batch 0,
cores 4-7 batch 1). Two ncfw collectives, both triggered as early as
possible:
  AR1: all-8 AllReduce (2KB bf16) of the global row-sum partials of the
       *LN-normalized* q,k. Computed straight from the raw input tiles
       plus per-row LN stats via tiny PE matmuls
       (sum_n rstd_n*x[n,:] + sum_n(-m_n*rstd_n)), i.e. without waiting
       for the normalize/transpose pipeline -- trigger at ~t=18us.
       The head-mean projection (@W_eff + bias) runs after the reduce.
  AR2: per-batch-group ([[0-3],[4-7]]) AllReduce of the S partials
       (bf16, 128KB). The normalize/transpose/projection pipeline and
       the weight-predictor MLP overlap the collective flights.
Weights are replicated; W_in/W_out are pre-cast to bf16 on the host
(they were cast on-device before, so numerics are unchanged).
"""

import os
import numpy as np

H, DH, B, N, D = 8, 64, 2, 2048, 512
EPS = 1e-5
N_CORES = 8
R = (B * N) // N_CORES  # rows per core = 512
P = 128                 # SBUF partitions

WARM_A = int(os.environ.get("BASSK_WARM_A", "60"))  # upfront PE clock ramp
WARM_B = int(os.environ.get("BASSK_WARM_B", "16"))  # pre-S bridge
WARM_C = int(os.environ.get("BASSK_WARM_C", "48"))  # pre-tail bridge
WARM_LN = int(os.environ.get("BASSK_WARM_LN", "0"))  # per-row-tile drumbeat
PRIME = os.environ.get("BASSK_PRIME", "1") == "1"   # early cc-stream wakeup

_CACHE = {}


def _build_program():
    import concourse.bacc as bacc
    import concourse.bass as bass
    import concourse.mybir as mybir
    import concourse.tile as tile
    from concourse.masks import make_identity
    from contextlib import ExitStack

    f32 = mybir.dt.float32
    bf16 = mybir.dt.bfloat16
    CDT = bf16
    AX = mybir.AxisListType
    OP = mybir.AluOpType
    AF = mybir.ActivationFunctionType

    nc = bacc.Bacc("TRN2", target_bir_lowering=False, debug=False,
                   enable_asserts=True, num_devices=N_CORES)

    # ---- external I/O (per-core shapes) ----
    q_ext = nc.dram_tensor("q", [R, D], f32, kind="ExternalInput")
    k_ext = nc.dram_tensor("k", [R, D], f32, kind="ExternalInput")
    v_ext = nc.dram_tensor("v", [R, D], f32, kind="ExternalInput")
    winT_ext = nc.dram_tensor("W_inT", [D, D], bf16, kind="ExternalInput")    # [d, j]
    woutT_ext = nc.dram_tensor("W_outT", [D, D], bf16, kind="ExternalInput")  # [j, dcol]
    lng_ext = nc.dram_tensor("ln_g", [D], f32, kind="ExternalInput")
    lnb_ext = nc.dram_tensor("ln_b", [D], f32, kind="ExternalInput")
    bout_ext = nc.dram_tensor("b_out", [D], f32, kind="ExternalInput")
    w1T_ext = nc.dram_tensor("wp_w1T", [2 * DH, DH], f32, kind="ExternalInput")
    b1_ext = nc.dram_tensor("wp_b1", [DH], f32, kind="ExternalInput")
    wlg_ext = nc.dram_tensor("wp_ln_g", [DH], f32, kind="ExternalInput")
    wlb_ext = nc.dram_tensor("wp_ln_b", [DH], f32, kind="ExternalInput")
    w2_ext = nc.dram_tensor("wp_w2", [DH], f32, kind="ExternalInput")
    b2_ext = nc.dram_tensor("wp_b2", [1], f32, kind="ExternalInput")
    out_ext = nc.dram_tensor("out", [D, R], f32, kind="ExternalOutput")  # transposed
    DEBUG = os.environ.get("BASSK_DEBUG", "0") == "1"
    if DEBUG:
        dbg_ar1 = nc.dram_tensor("dbg_ar1", [P, 8], mybir.dt.bfloat16, kind="ExternalOutput")
        dbg_feat = nc.dram_tensor("dbg_feat", [2 * DH, H], mybir.dt.bfloat16, kind="ExternalOutput")
        dbg_w = nc.dram_tensor("dbg_w", [H, 1], f32, kind="ExternalOutput")
        dbg_ssum = nc.dram_tensor("dbg_ssum", [P, H * DH], mybir.dt.bfloat16, kind="ExternalOutput")

    NT = R // P  # row tiles per core = 4
    ND = D // P  # feature tiles = 4

    def _patch_pool(pool):
        orig = pool.tile
        def tile_(shape, dtype, tag, **kw):
            return orig(shape, dtype, name=tag, tag=tag, **kw)
        pool.tile = tile_
        return pool

    with tile.TileContext(nc) as tc, ExitStack() as ctx:
        consts = _patch_pool(ctx.enter_context(tc.tile_pool(name="consts", bufs=1)))
        wp = _patch_pool(ctx.enter_context(tc.tile_pool(name="wp", bufs=1)))
        work = _patch_pool(ctx.enter_context(tc.tile_pool(name="work", bufs=4)))
        keep = _patch_pool(ctx.enter_context(tc.tile_pool(name="keep", bufs=1)))
        psum = _patch_pool(ctx.enter_context(tc.tile_pool(name="psum", bufs=1, space="PSUM")))
        pss = _patch_pool(ctx.enter_context(tc.tile_pool(name="pss", bufs=1, space="PSUM")))
        dram = _patch_pool(ctx.enter_context(tc.tile_pool(name="dram", bufs=1, space="DRAM")))

        # ---------------- constants ----------------
        ident = consts.tile([P, P], CDT, tag="ident")
        make_identity(nc, ident[:])
        ones_row = consts.tile([1, P], CDT, tag="ones_row")      # K=1 bias matmul lhsT
        nc.vector.memset(ones_row[:], 1.0)
        ones_row_f = consts.tile([1, P], f32, tag="ones_row_f")
        nc.vector.memset(ones_row_f[:], 1.0)
        ones_col_bf = consts.tile([P, 1], CDT, tag="ones_col_bf")
        nc.vector.memset(ones_col_bf[:], 1.0)
        neg1_row64 = consts.tile([1, DH], CDT, tag="neg1_row64")
        nc.vector.memset(neg1_row64[:], -1.0)
        eps_t = consts.tile([P, 1], f32, tag="eps_t")
        nc.vector.memset(eps_t[:], EPS)
        ind_top = consts.tile([1, P], bf16, tag="ind_top")
        nc.vector.memset(ind_top[:], 0.0)
        nc.vector.memset(ind_top[:, 0:DH], 1.0)
        ind_bot = consts.tile([1, P], bf16, tag="ind_bot")
        nc.vector.memset(ind_bot[:], 0.0)
        nc.vector.memset(ind_bot[:, DH:P], 1.0)
        one1 = consts.tile([1, 1], CDT, tag="one1")
        nc.vector.memset(one1[:], 1.0)
        ident8 = consts.tile([H, H], f32, tag="ident8")
        make_identity(nc, ident8[:])

        # prime the ncfw collective stream: absorbs the ~30us first-trigger
        # wakeup + entry barrier while local compute proceeds
        if PRIME:
            cc_prime = consts.tile([1, 64], bf16, tag="cc_prime")
            nc.vector.memset(cc_prime[:], 0.0)
            prime_in = dram.tile([64], bf16, tag="prime_in")
            prime_out = dram.tile([64], bf16, tag="prime_out")
            nc.gpsimd.dma_start(prime_in[:].unsqueeze(0), cc_prime[:])
            nc.gpsimd.collective_compute(
                "AllReduce", OP.add,
                replica_groups=[[0, 1], [2, 3], [4, 5], [6, 7]],
                ins=[prime_in.opt()], outs=[prime_out.opt()])

        # pre-load ACT function tables off the critical path
        tbl_sink = consts.tile([1, 1], f32, tag="tbl_sink")
        for fn in (AF.Identity, AF.Square, AF.Sqrt, AF.Sigmoid, AF.Relu):
            nc.scalar.activation(tbl_sink[:], eps_t[0:1, :], fn)

        # PE clock warm-up: dense matmuls during the initial DMA window
        warm_sink = consts.tile([1, 1], f32, tag="warm_sink")
        if WARM_A:
            warm_ps = psum.tile([P, P], f32, tag="trps", bufs=3)
            for wi in range(WARM_A):
                nc.tensor.matmul(warm_ps[:], ident[:], ident[:],
                                 start=True, stop=True)
            nc.vector.tensor_copy(warm_sink[:], warm_ps[0:1, 0:1])

        # ---------------- phase 1: LN + normalize + AR1 row-sums ----------
        def ln_stats(x_sb):
            """bn stats -> r2 [P, 2] = [rstd | -m*rstd]."""
            st6 = work.tile([P, 6], f32, tag="st6")
            nc.vector.bn_stats(st6[:], x_sb[:])
            mv = work.tile([P, 2], f32, tag="mv")
            nc.vector.bn_aggr(mv[:], st6[:])
            r2 = work.tile([P, 2], f32, tag="r2", bufs=6)
            nc.scalar.activation(r2[:, 0:1], mv[:, 1:2], AF.Sqrt, bias=eps_t[:])
            nc.vector.reciprocal(r2[:, 0:1], r2[:, 0:1])
            nc.vector.tensor_scalar(r2[:, 1:2], mv[:, 0:1], scalar1=-1.0,
                                    scalar2=r2[:, 0:1], op0=OP.mult, op1=OP.mult)
            return r2

        def normalize(x_sb, r2, rt, name):
            xn = keep.tile([P, D], CDT, tag=f"xn_{name}")
            if rt % 2 == 0:
                nc.scalar.activation(xn[:], x_sb[:], AF.Identity,
                                     bias=r2[:, 1:2], scale=r2[:, 0:1])
            else:
                nc.vector.tensor_scalar(xn[:], x_sb[:], scalar1=r2[:, 0:1],
                                        scalar2=r2[:, 1:2],
                                        op0=OP.mult, op1=OP.add)
            return xn

        ps_xbq = pss.tile([P, ND], f32, tag="pssA")
        ps_xbk = pss.tile([P, ND], f32, tag="pssB")
        xn_q, xn_k = [], []

        def phase1(x_ext, name, dmae, xn_list, ps_xb):
            for rt in range(NT):
                x = work.tile([P, D], f32, tag=f"x_{name}", bufs=3)
                dmae.dma_start(x[:], x_ext[rt * P:(rt + 1) * P, :])
                r2 = ln_stats(x)
                xn = normalize(x, r2, rt, f"{name}{rt}")
                xn_list.append(xn)
                for di in range(ND):
                    nc.tensor.matmul(
                        ps_xb[:, di:di + 1],
                        xn[:, di * P:(di + 1) * P],
                        ones_col_bf[:],
                        start=(rt == 0), stop=(rt == NT - 1))

        phase1(q_ext, "q", nc.sync, xn_q, ps_xbq)
        phase1(k_ext, "k", nc.scalar, xn_k, ps_xbk)

        ar1_sb = keep.tile([P, 2 * ND], bf16, tag="ar1_sb")
        nc.vector.tensor_scalar_mul(ar1_sb[:, 0:ND], ps_xbq[:], 1.0 / (B * N))
        nc.scalar.activation(ar1_sb[:, ND:2 * ND], ps_xbk[:], AF.Identity,
                             scale=1.0 / (B * N))
        AR1EL = P * 2 * ND
        ar1_in = dram.tile([AR1EL], bf16, tag="ar1_in")
        ar1_out = dram.tile([AR1EL], bf16, tag="ar1_out")
        nc.sync.dma_start(ar1_in[:].rearrange("(p f) -> p f", p=P), ar1_sb[:])
        nc.gpsimd.collective_compute(
            "AllReduce", OP.add,
            replica_groups=[list(range(N_CORES))],
            ins=[ar1_in.opt()], outs=[ar1_out.opt()])

        # ---------------- weights (overlap AR1 flight) ----------------
        winT = []
        weff = []
        bsl_bf = []
        for di in range(ND):
            wt = wp.tile([P, D], bf16, tag=f"winT{di}")
            nc.scalar.dma_start(wt[:], winT_ext[di * P:(di + 1) * P, :])
            winT.append(wt)
            g = wp.tile([P, 1], f32, tag=f"gsl{di}")
            nc.scalar.dma_start(g[:], lng_ext[di * P:(di + 1) * P].unsqueeze(1))
            b = wp.tile([P, 1], f32, tag=f"bsl{di}")
            nc.scalar.dma_start(b[:], lnb_ext[di * P:(di + 1) * P].unsqueeze(1))
            bb = wp.tile([P, 1], bf16, tag=f"bslb{di}")
            nc.vector.tensor_copy(bb[:], b[:])
            bsl_bf.append(bb)
            we = wp.tile([P, D], CDT, tag=f"weff{di}")
            nc.vector.tensor_scalar_mul(we[:], wt[:], g[:])  # W_in^T * g
            weff.append(we)

        # bias_row[1, j] = ln_b @ W_in^T  (rank-1 LN-bias term)
        bias_ps = psum.tile([1, D], f32, tag="projps", bufs=2)
        for di in range(ND):
            nc.tensor.matmul(bias_ps[:], bsl_bf[di][:], winT[di][:],
                             start=(di == 0), stop=(di == ND - 1))
        bias_row = wp.tile([1, D], CDT, tag="bias_row")
        nc.scalar.copy(bias_row[:], bias_ps[:])
        # bias_rowT [DH, H]: bias_rowT[c, h] = bias_row[h*DH+c]
        brT_ps = psum.tile([DH, H], f32, tag="projps", bufs=2)
        for di in range(ND):
            for h in range(H):
                nc.tensor.matmul(
                    brT_ps[:, h:h + 1],
                    winT[di][:, h * DH:(h + 1) * DH],
                    bsl_bf[di][:],
                    start=(di == 0), stop=(di == ND - 1))
        bias_rT = wp.tile([DH, H], f32, tag="bias_rT")
        nc.vector.tensor_copy(bias_rT[:], brT_ps[:])

        # weight-predictor weights
        w1T = wp.tile([2 * DH, DH], f32, tag="w1T")
        nc.gpsimd.dma_start(w1T[:], w1T_ext[:])
        w1T_bf = wp.tile([2 * DH, DH], bf16, tag="w1T_bf")
        nc.vector.tensor_copy(w1T_bf[:], w1T[:])
        b1_rep = wp.tile([H, DH], f32, tag="b1_rep")
        nc.gpsimd.dma_start(b1_rep[:], b1_ext[None, :].to_broadcast((H, DH)))
        wlg_rep = wp.tile([H, DH], f32, tag="wlg_rep")
        nc.gpsimd.dma_start(wlg_rep[:], wlg_ext[None, :].to_broadcast((H, DH)))
        wlb_rep = wp.tile([H, DH], f32, tag="wlb_rep")
        nc.gpsimd.dma_start(wlb_rep[:], wlb_ext[None, :].to_broadcast((H, DH)))
        w2_rep = wp.tile([H, DH], f32, tag="w2_rep")
        nc.gpsimd.dma_start(w2_rep[:], w2_ext[None, :].to_broadcast((H, DH)))
        b2_col = wp.tile([H, 1], f32, tag="b2_col")
        nc.gpsimd.dma_start(b2_col[:], b2_ext[None, :].to_broadcast((H, 1)))

        # W_out (tail-only weights)
        woutT = []
        bout = []
        for jt in range(ND):
            wo = wp.tile([P, D], CDT, tag=f"woutT{jt}")
            nc.gpsimd.dma_start(wo[:], woutT_ext[jt * P:(jt + 1) * P, :])
            woutT.append(wo)
            bo = wp.tile([P, 1], f32, tag=f"bout{jt}")
            nc.gpsimd.dma_start(bo[:], bout_ext[jt * P:(jt + 1) * P].unsqueeze(1))
            bout.append(bo)

        # ---------------- per-tensor pipeline helpers ----------------
        def transpose_xn(xn):
            if WARM_LN:
                wps = psum.tile([P, P], f32, tag="trps", bufs=3)
                for wi in range(WARM_LN):
                    nc.tensor.matmul(wps[:], ident[:], ident[:],
                                     start=True, stop=True)
            tr_ps = psum.tile([P, D], CDT, tag="trps", bufs=3)
            for di in range(ND):
                nc.tensor.transpose(
                    tr_ps[:, di * P:(di + 1) * P],
                    xn[:, di * P:(di + 1) * P], ident[:])
            return tr_ps

        def project(xnT_ps_view, name, rt, act_copy):
            """fx[rt] = xn @ (W_in*g)^T + ln_b @ W_in^T; returns psum tile."""
            # xnT_ps_view: [P(d), (di, P rows)] psum from transpose; must be
            # copied to SBUF first (matmul lhsT reads SBUF).
            xnT = work.tile([P, D], CDT, tag="xnT_sb", bufs=6)
            if rt % 2 == 0:
                nc.vector.tensor_copy(xnT[:], xnT_ps_view)
            else:
                nc.scalar.copy(xnT[:], xnT_ps_view)
            pj = psum.tile([P, D], f32, tag="projps", bufs=2)
            for di in range(ND):
                nc.tensor.matmul(
                    pj[:], xnT[:, di * P:(di + 1) * P], weff[di][:],
                    start=(di == 0), stop=False)
            nc.tensor.matmul(pj[:], ones_row[:], bias_row[:],
                             start=False, stop=True)
            return pj

        # ---- interleaved k/v/q row-tile pipeline ----
        def rowstats_A(pj, rt, keep_A_tag=None):
            """fx copy + per-head inv-norm / mean; A = [cos | centered]."""
            fx = work.tile([P, D], CDT, tag="fx", bufs=4)
            if rt % 2 == 0:
                nc.scalar.copy(fx[:], pj[:])
            else:
                nc.vector.tensor_copy(fx[:], pj[:])
            fx3 = fx[:].rearrange("p (h c) -> p h c", h=H)
            sqh = work.tile([P, D], CDT, tag="sqh")
            nc.scalar.activation(sqh[:], pj[:], AF.Square)
            qn2 = work.tile([P, H], f32, tag="qn2")
            nc.vector.reduce_sum(
                qn2[:], sqh[:].rearrange("p (h c) -> p h c", h=H), axis=AX.X)
            qsum = work.tile([P, H], f32, tag="qsum")
            nc.vector.reduce_sum(qsum[:], fx3, axis=AX.X)
            invn = work.tile([P, H], f32, tag="invn")
            nc.scalar.activation(invn[:], qn2[:], AF.Sqrt)
            nc.vector.reciprocal(invn[:], invn[:])
            hmean = work.tile([P, H], f32, tag="hmean")
            nc.vector.tensor_scalar_mul(hmean[:], qsum[:], 1.0 / DH)
            if keep_A_tag is not None:
                A = keep.tile([P, 2 * D], CDT, tag=keep_A_tag)
            else:
                A = work.tile([P, 2 * D], CDT, tag="A_q", bufs=2)
            A4 = A[:].rearrange("p (h c) -> p h c", h=H)
            nc.vector.tensor_tensor(
                A4[:, :, 0:DH], fx3,
                invn[:, :, None].broadcast_to((P, H, DH)), op=OP.mult)
            nc.vector.tensor_tensor(
                A4[:, :, DH:2 * DH], fx3,
                hmean[:, :, None].broadcast_to((P, H, DH)), op=OP.subtract)
            return A

        Ak = [None] * NT
        fv_tiles = [None] * NT
        AqT_all = keep.tile([P, H * R], CDT, tag="AqT_all")
        AqT8 = AqT_all[:].rearrange("p (h r) -> p h r", h=H)

        def k_tile(rt):
            tr_ps = transpose_xn(xn_k[rt])
            pj = project(tr_ps[:], "k", rt, act_copy=True)
            Ak[rt] = rowstats_A(pj, rt, keep_A_tag=f"A_k{rt}")

        def v_tile(rt):
            xv = work.tile([P, D], f32, tag="x_v", bufs=3)
            nc.sync.dma_start(xv[:], v_ext[rt * P:(rt + 1) * P, :])
            r2 = ln_stats(xv)
            xnv = normalize(xv, r2, rt, f"v{rt}")
            tr_ps = transpose_xn(xnv)
            pj = project(tr_ps[:], "v", rt, act_copy=True)
            fv = keep.tile([P, D], CDT, tag=f"fv{rt}")
            if rt % 2 == 0:
                nc.scalar.copy(fv[:], pj[:])
            else:
                nc.vector.tensor_copy(fv[:], pj[:])
            fv_tiles[rt] = fv

        def q_tile(rt):
            tr_ps = transpose_xn(xn_q[rt])
            pj = project(tr_ps[:], "q", rt, act_copy=True)
            A = rowstats_A(pj, rt)
            for hg in range(2):
                aq_ps = psum.tile([P, D], CDT, tag="trps", bufs=3)
                for hh in range(4):
                    h = hg * 4 + hh
                    nc.tensor.transpose(
                        aq_ps[:, hh * P:(hh + 1) * P],
                        A[:, h * 2 * DH:(h + 1) * 2 * DH], ident[:])
                dst = AqT8[:, hg * 4:hg * 4 + 4, rt * P:(rt + 1) * P]
                srcv = aq_ps[:].rearrange("p (hh r) -> p hh r", hh=4)
                if hg == 0:
                    nc.vector.tensor_copy(dst, srcv)
                else:
                    nc.scalar.copy(dst, srcv)

        for rt in range(NT):
            k_tile(rt)
            v_tile(rt)
            if rt >= 1:
                q_tile(rt - 1)

        if WARM_B:
            warm3_ps = psum.tile([P, P], f32, tag="trps", bufs=3)
            for wi in range(WARM_B):
                nc.tensor.matmul(warm3_ps[:], ident[:], fv_tiles[3][:, 0:P],
                                 start=True, stop=True)
            nc.vector.tensor_copy(warm_sink[:], warm3_ps[0:1, 0:1])

        # ---- S partials ----
        s_ps = pss.tile([P, H * DH], f32, tag="pssA")
        for h in range(H):
            for rt in range(NT):
                nc.tensor.matmul(
                    s_ps[:, h * DH:(h + 1) * DH],
                    Ak[rt][:, h * 2 * DH:(h + 1) * 2 * DH],
                    fv_tiles[rt][:, h * DH:(h + 1) * DH],
                    start=(rt == 0), stop=(rt == NT - 1))
        s_bf = keep.tile([P, H * DH], bf16, tag="s_bf")
        nc.vector.tensor_copy(s_bf[:], s_ps[:])
        SEL = P * H * DH
        ar2_in = dram.tile([SEL], bf16, tag="ar2_in")
        ar2_out = dram.tile([SEL], bf16, tag="ar2_out")
        nc.sync.dma_start(ar2_in[:].rearrange("(p f) -> p f", p=P), s_bf[:])
        nc.gpsimd.collective_compute(
            "AllReduce", OP.add,
            replica_groups=[[0, 1, 2, 3], [4, 5, 6, 7]],
            ins=[ar2_in.opt()], outs=[ar2_out.opt()])

        # last q tile overlaps AR2 flight
        q_tile(NT - 1)

        # ---- weight-predictor MLP (needs AR1; overlaps AR2 flight) ----
        xg_sb = keep.tile([P, 2 * ND], bf16, tag="xg_sb")
        nc.scalar.dma_start(xg_sb[:], ar1_out[:].rearrange("(p f) -> p f", p=P))
        featq_ps = psum.tile([DH, H], f32, tag="trps", bufs=3)
        featk_ps = psum.tile([DH, H], f32, tag="trps", bufs=3)
        for di in range(ND):
            for h in range(H):
                nc.tensor.matmul(
                    featq_ps[:, h:h + 1],
                    weff[di][:, h * DH:(h + 1) * DH],
                    xg_sb[:, di:di + 1],
                    start=(di == 0), stop=(di == ND - 1))
                nc.tensor.matmul(
                    featk_ps[:, h:h + 1],
                    weff[di][:, h * DH:(h + 1) * DH],
                    xg_sb[:, ND + di:ND + di + 1],
                    start=(di == 0), stop=(di == ND - 1))
        featT = keep.tile([2 * DH, H], bf16, tag="featT")
        nc.vector.tensor_tensor(featT[0:DH, :], featq_ps[:], bias_rT[:], op=OP.add)
        nc.vector.tensor_tensor(featT[DH:2 * DH, :], featk_ps[:], bias_rT[:], op=OP.add)

        hid_ps = psum.tile([H, DH], f32, tag="trps", bufs=3)
        nc.tensor.matmul(hid_ps[:], featT[:], w1T_bf[:], start=True, stop=True)
        hid = keep.tile([H, DH], f32, tag="hid")
        nc.vector.tensor_tensor(hid[:], hid_ps[:], b1_rep[:], op=OP.add)
        hst6 = keep.tile([H, 6], f32, tag="hst6")
        nc.vector.bn_stats(hst6[:], hid[:])
        hmv = keep.tile([H, 2], f32, tag="hmv")
        nc.vector.bn_aggr(hmv[:], hst6[:])
        hrstd = keep.tile([H, 1], f32, tag="hrstd")
        nc.scalar.activation(hrstd[:], hmv[:, 1:2], AF.Sqrt, bias=eps_t[0:H, :])
        nc.vector.reciprocal(hrstd[:], hrstd[:])
        hln = keep.tile([H, DH], f32, tag="hln")
        nc.vector.tensor_scalar(hln[:], hid[:], scalar1=hmv[:, 0:1],
                                scalar2=hrstd[:], op0=OP.subtract, op1=OP.mult)
        nc.vector.tensor_tensor(hln[:], hln[:], wlg_rep[:], op=OP.mult)
        nc.vector.tensor_tensor(hln[:], hln[:], wlb_rep[:], op=OP.add)
        nc.scalar.activation(hln[:], hln[:], AF.Relu)
        lscr = keep.tile([H, DH], f32, tag="lscr")
        nc.vector.tensor_tensor(lscr[:], hln[:], w2_rep[:], op=OP.mult)
        logit = keep.tile([H, 1], f32, tag="logit")
        nc.vector.reduce_sum(logit[:], lscr[:], axis=AX.X)
        wcol = keep.tile([H, 1], f32, tag="wcol")
        nc.scalar.activation(wcol[:], logit[:], AF.Sigmoid, bias=b2_col[:])
        wr_ps = psum.tile([1, H], f32, tag="trps", bufs=3)
        nc.tensor.transpose(wr_ps[:], wcol[:], ident8[:])
        wrow = keep.tile([1, H], f32, tag="wrow")
        nc.vector.tensor_copy(wrow[:], wr_ps[:])
        omw = keep.tile([1, H], bf16, tag="omw")
        nc.vector.tensor_scalar(omw[:], wrow[:], scalar1=-1.0, scalar2=1.0,
                                op0=OP.mult, op1=OP.add)
        wdh = keep.tile([1, H], bf16, tag="wdh")
        nc.vector.tensor_scalar_mul(wdh[:], wrow[:], 1.0 / DH)
        wsc_ps = psum.tile([P, H], f32, tag="trps", bufs=3)
        nc.tensor.matmul(wsc_ps[:], ind_top[:], omw[:], start=True, stop=False)
        nc.tensor.matmul(wsc_ps[:], ind_bot[:], wdh[:], start=False, stop=True)
        wsc = keep.tile([P, H], bf16, tag="wsc")
        nc.vector.tensor_copy(wsc[:], wsc_ps[:])

        # ---- S readback, blend-scale, final projection ----
        s_sum = keep.tile([P, H * DH], bf16, tag="s_sum")
        HSEL = SEL // 2
        nc.sync.dma_start(
            s_sum[0:P // 2, :], ar2_out[0:HSEL].rearrange("(p f) -> p f", p=P // 2))
        nc.scalar.dma_start(
            s_sum[P // 2:P, :],
            ar2_out[HSEL:SEL].rearrange("(p f) -> p f", p=P // 2))
        if WARM_C:
            warm2_ps = psum.tile([P, P], f32, tag="trps", bufs=3)
            for wi in range(WARM_C):
                nc.tensor.matmul(warm2_ps[:], ident[:], ident[:],
                                 start=True, stop=True)
            nc.vector.tensor_copy(warm_sink[:], warm2_ps[0:1, 0:1])
        if DEBUG:
            nc.gpsimd.dma_start(dbg_ar1[:], xg_sb[:])
            nc.gpsimd.dma_start(dbg_feat[:], featT[:])
            nc.gpsimd.dma_start(dbg_w[:], wcol[:])
            nc.gpsimd.dma_start(dbg_ssum[:], s_sum[:])
        s_sc = keep.tile([P, H * DH], CDT, tag="s_sc")
        nc.vector.tensor_tensor(
            s_sc[:].rearrange("p (h c) -> p h c", h=H),
            s_sum[:].rearrange("p (h c) -> p h c", h=H),
            wsc[:, :, None].broadcast_to((P, H, DH)), op=OP.mult)

        foutT = []
        for jt in range(ND):
            ft = keep.tile([P, R], CDT, tag=f"foutT{jt}")
            foutT.append(ft)
        for h in range(H):
            m_ps = psum.tile([DH, R], f32, tag="projps", bufs=2)
            nc.tensor.matmul(m_ps[:], s_sc[:, h * DH:(h + 1) * DH],
                             AqT_all[:, h * R:(h + 1) * R],
                             start=True, stop=True)
            dst = foutT[h // 2][(h % 2) * DH:(h % 2) * DH + DH, :]
            if h % 2 == 0:
                nc.scalar.copy(dst, m_ps[:])
            else:
                nc.vector.tensor_copy(dst, m_ps[:])

        _ldq = [nc.sync, nc.scalar]
        for dt_ in range(ND):
            o_ps = psum.tile([P, R], f32, tag="projps", bufs=2)
            for jt in range(ND):
                nc.tensor.matmul(
                    o_ps[:], woutT[jt][:, dt_ * P:(dt_ + 1) * P], foutT[jt][:],
                    start=(jt == 0), stop=(jt == ND - 1))
            o_sb = work.tile([P, R], f32, tag="o_sb")
            if dt_ % 2 == 0:
                nc.scalar.activation(o_sb[:], o_ps[:], AF.Identity,
                                     bias=bout[dt_][:], scale=1.0)
            else:
                nc.vector.tensor_scalar_add(o_sb[:], o_ps[:], bout[dt_][:])
            _ldq[dt_ % 2].dma_start(out_ext[dt_ * P:(dt_ + 1) * P, :], o_sb[:])

    nc.finalize()
    return nc


def _get_program():
    if "nc" not in _CACHE:
        _CACHE["nc"] = _build_program()
    return _CACHE["nc"]


def _make_in_maps(inputs):
    import ml_dtypes
    bf = ml_dtypes.bfloat16
    q = np.ascontiguousarray(np.asarray(inputs["q"], np.float32).reshape(B * N, D))
    k = np.ascontiguousarray(np.asarray(inputs["k"], np.float32).reshape(B * N, D))
    v = np.ascontiguousarray(np.asarray(inputs["v"], np.float32).reshape(B * N, D))
    shared = {
        "W_inT": np.ascontiguousarray(np.asarray(inputs["W_in"], np.float32).T.astype(bf)),
        "W_outT": np.ascontiguousarray(np.asarray(inputs["W_out"], np.float32).T.astype(bf)),
        "ln_g": np.asarray(inputs["ln_g"], np.float32),
        "ln_b": np.asarray(inputs["ln_b"], np.float32),
        "b_out": np.asarray(inputs["b_out"], np.float32),
        "wp_w1T": np.ascontiguousarray(np.asarray(inputs["wp_w1"], np.float32).T),
        "wp_b1": np.asarray(inputs["wp_b1"], np.float32),
        "wp_ln_g": np.asarray(inputs["wp_ln_g"], np.float32),
        "wp_ln_b": np.asarray(inputs["wp_ln_b"], np.float32),
        "wp_w2": np.ascontiguousarray(np.asarray(inputs["wp_w2"], np.float32).reshape(DH)),
        "wp_b2": np.asarray(inputs["wp_b2"], np.float32).reshape(1),
    }
    in_maps = []
    for c in range(N_CORES):
        m = dict(shared)
        sl = slice(c * R, (c + 1) * R)
        m["q"] = np.ascontiguousarray(q[sl])
        m["k"] = np.ascontiguousarray(k[sl])
        m["v"] = np.ascontiguousarray(v[sl])
        in_maps.append(m)
    return in_maps


def _gather(results):
    out = np.empty((B * N, D), np.float32)
    for c in range(N_CORES):
        out[c * R:(c + 1) * R, :] = results[c]["out"].T
    return out.reshape(B, N, D)


def _run(inputs, trace=False, trace_cores=None):
    from concourse.bass_utils import run_bass_kernel_spmd
    nc = _get_program()
    in_maps = _make_in_maps(inputs)
    res = run_bass_kernel_spmd(
        nc, in_maps, core_ids=list(range(N_CORES)),
        trace=trace, trace_cores=trace_cores)
    return _gather(res.results), res


def kernel(**inputs) -> np.ndarray:
    out, _ = _run(inputs, trace=False)
    return out


def run_traced(inputs, trace_cores=None):
    return _run(inputs, trace=True, trace_cores=trace_cores)
